# revision 31
# baseline (speedup 1.0000x reference)
"""CustomLSTMCell fused kernel for 8x Trainium2 NeuronCores.

Reference computation (B=8192, D=H=1024):
    z = e_t @ W_x.T + h_prev @ W_h.T + (b_x + b_h + b_extra)   # [B, 4H]
    f, i, o, c = split(z, 4)
    c_t = sigmoid(f) * c_prev + sigmoid(i) * tanh(c)
    h_t = sigmoid(o) * tanh(c_t)

Sharding: 2-way batch x 4-way hidden-unit (8 cores, no collectives).
Each core computes z transposed ([gate_rows, batch] layout) so the bias
folds into the ScalarE activation's per-partition bias operand, and both
matmul operands arrive pre-transposed from the host (contraction dim on
partitions).

v2 (this version, ~244-247us HW vs the f32r baseline's ~259us): matmul
operands and h/c outputs in float16 (PSUM accumulation stays fp32;
rel-max-err 6.8e-4, tolerance 2e-2).  HW-measured on this chip: a dense
k-accumulating MATMUL stream runs ~219 ns/MM at 16-bit vs 232.4 ns/MM
at float32r (LDWEIGHTS is hidden by the PE reorder window in both
cases, so the win is the stream rate itself, not weight loads).  fp8
DoubleRow measures 2x (not the cost model's 4x), and plain-e4m3
precision is 4.3x over the error budget (rel-max-err 8.6e-2), so fp8
multi-pass hi/lo schemes all lose to a single 16-bit pass.  fp16 over
bf16 for the extra mantissa.  Moving free dim >512 is rejected by the
ISA (s3d3_mm_num_elements) even at 16-bit.

Measured budget of a good run: ~8us entry butterfly (engines stagger up
0.4-7.4us, DMA queues release ~8.7), warmups to 13.1, 221.5us PE-bound
MM stream (98.6% dense), ~1.5us post-stream elementwise tail, ~4.5us
counted epilogue.  Run-to-run noise is +-2.5us; occasional ~292us runs
are P0 power-state downclock (PE at 2.0GHz), environmental.

Schedule notes:
 - Chase phase: W streams per-k-chunk on the sync HWDGE ring while the
   first batch-block's x streams on gpsimd; the first batch block runs
   its matmuls k-outer so the PE starts as soon as chunk 0 lands and
   chases the arrival stream.  Both rings are up early enough (~2us /
   ~6.2us) to pre-queue descriptors before the DMA queues release
   (~8.7us), and two rings halve the per-chunk delivery time — PE
   chase gaps drop from ~3.3us to ~0 (the scalar ring, up at 7.2us,
   could NOT pre-queue and starved the chase by +3.6us instead).
 - Steady-state x arrives as one whole-block 2MB DMA (1KB lines), one
   completion sem per 128 matmuls, issued 2+ blocks ahead.
 - Each DMA_DIRECT2D costs ~650ns of issue time on its engine, and a DMA
   chain on the scalar engine develops issue-blocking sem waits that
   delay ACTIVATEs (which gate PSUM-bank release).  So: bulk input
   stream on sync, cprev + steady-state outputs on gpsimd (SWDGE), and
   the scalar ring carries only the bias load and the final h store.
 - The final quadruple runs each gate as two half-width accumulation
   groups in (c,i,f,o) order, elementwise ops stage-batched across the
   halves, so the post-last-matmul chain is only ACT(o)+mult+store of
   the second half (~1.5us).
"""

import sys

if "/opt/trn_rl_repo" not in sys.path:
    sys.path.insert(0, "/opt/trn_rl_repo")

import numpy as np

import concourse.bass as bass
import concourse.mybir as mybir
from concourse import bacc
from concourse.bass_utils import run_bass_kernel_spmd
from concourse.tile import TileContext

F32 = mybir.dt.float32
F16 = mybir.dt.float16
AFT = mybir.ActivationFunctionType
ALU = mybir.AluOpType

B, D, H = 8192, 1024, 1024
M_BATCH, M_UNIT = 2, 4          # batch split x unit split = 8 cores
BS = B // M_BATCH               # 4096 batch rows per core
U = H // M_UNIT                 # 256 hidden units per core
K = D + H                       # 2048 contraction (e_t | h_prev)
KT = K // 128                   # 16 k-chunks
G = 4 * U                       # 1024 gate rows per core (f|i|o|c x U)
BBLK = 512                      # moving free-dim per matmul
NBB = BS // BBLK                # 8 batch blocks
NJ = U // 128                   # 2 unit sub-blocks of 128 partitions

GATE_FUNCS = [AFT.Sigmoid, AFT.Sigmoid, AFT.Sigmoid, AFT.Tanh]  # f, i, o, c


def _build_nc():
    nc = bacc.Bacc()

    xT = nc.dram_tensor("xT", [K, BS], F16, kind="ExternalInput")
    wT = nc.dram_tensor("wT", [K, G], F16, kind="ExternalInput")
    bias = nc.dram_tensor("bias", [G], F32, kind="ExternalInput")
    cT = nc.dram_tensor("cT", [U, BS], F32, kind="ExternalInput")
    hT_out = nc.dram_tensor("hT_out", [U, BS], F16, kind="ExternalOutput")
    cT_out = nc.dram_tensor("cT_out", [U, BS], F16, kind="ExternalOutput")

    xT_r = xT.ap().rearrange("(k p) b -> p k b", p=128)      # [128, KT, BS]
    wT_r = wT.ap().rearrange("(k p) g -> p k g", p=128)      # [128, KT, G]
    bias_r = bias.ap().rearrange("(c p) -> p c", p=128)      # [128, 4*NJ]
    cT_r = cT.ap().rearrange("(j p) b -> p j b", p=128)      # [128, NJ, BS]
    hT_r = hT_out.ap().rearrange("(j p) b -> p j b", p=128)
    cTo_r = cT_out.ap().rearrange("(j p) b -> p j b", p=128)

    with TileContext(nc) as tc:
        with (
            tc.tile_pool(name="wpool", bufs=1) as wpool,
            tc.tile_pool(name="xpool", bufs=2) as xpool,
            tc.tile_pool(name="cpool", bufs=2) as cpool,
            tc.tile_pool(name="gpool", bufs=2) as gpool,
            tc.tile_pool(name="opool", bufs=2) as opool,
            tc.tile_pool(name="psum", bufs=2, space="PSUM") as pp,
        ):
            bias_sb = wpool.tile([128, 4 * NJ], F32)
            nc.scalar.dma_start(out=bias_sb[:], in_=bias_r)

            # PE warm-up: ~12 throwaway matmuls on zeroed scratch while the
            # first W/x chunks are in flight (the DMA queues release at
            # ~8.7us after the entry butterfly; w0+x0k0 land ~9.7).  The
            # warmups run 7.9-13.1us, outlasting chunk arrival: a PE-idle
            # gap >3.4us between warmup-end and the stream would re-throttle
            # the HAM clock gate.  (Runs that measure ~292us with a uniform
            # ~263ns/MM stream are P0 power-state downclock to ~2.0GHz —
            # environmental, not schedule-dependent.)
            warm = wpool.tile([128, BBLK], F16, name="warm")
            nc.vector.memset(warm[:], 0.0)
            warm_ps = pp.tile([128, BBLK], F32, tag="ps0", name="warm_ps")
            # 8 warmups (8.0-11.4us) match chunk0's arrival once chunk 0 is
            # split into 6 descriptors (larger round-robin bandwidth share,
            # ~10us).  With 2-descriptor chunk0 (~12.3us arrival), 12
            # warmups were needed; 4 measured worse (PE idle 9.9-12.5).
            for _ in range(8):
                nc.tensor.matmul(
                    warm_ps[:], warm[:, 0:128], warm[:], start=True, stop=True
                )

            w_sb = []
            x0_sb = []
            for k in range(KT):
                wt = wpool.tile([128, G], F16, tag=f"w{k}", name=f"w{k}")
                if k == 0:
                    # chunk 0 split into more descriptors: pre-queued chase
                    # DMAs share the queues round-robin, so more descriptors
                    # = a larger bandwidth share = chunk 0 lands ~9.5-10us
                    # instead of ~12.3, letting the real stream start sooner.
                    for q in range(4):
                        nc.sync.dma_start(
                            out=wt[:, q * 256:(q + 1) * 256],
                            in_=wT_r[:, 0, q * 256:(q + 1) * 256],
                        )
                else:
                    nc.sync.dma_start(out=wt[:], in_=wT_r[:, k, :])
                w_sb.append(wt)
                if k % 2 == 0:
                    xp = xpool.tile(
                        [128, 2, BBLK], F16, tag=f"xp{k // 2}", name=f"xp{k // 2}", bufs=3
                    )
                # x0 on gpsimd (up at ~6.2us — early enough to pre-queue a
                # few descriptors before the queues release at ~8.7us),
                # leaving sync to stream W alone: two rings halve the chase
                # delivery time per k-chunk.  (The scalar ring, up at 7.2,
                # could NOT pre-queue and starved the chase by +3.6us.)
                if k == 0:
                    for q in range(2):
                        nc.gpsimd.dma_start(
                            out=xp[:, 0, q * 256:(q + 1) * 256],
                            in_=xT_r[:, 0, q * 256:(q + 1) * 256],
                        )
                else:
                    nc.gpsimd.dma_start(out=xp[:, k % 2, :], in_=xT_r[:, k, 0:BBLK])
                x0_sb.append(xp[:, k % 2, :])

            def load_cprev(bb):
                t = cpool.tile([128, NJ, BBLK], F32, tag="cprev", name="cprev")
                nc.gpsimd.dma_start(
                    out=t[:], in_=cT_r[:, :, bb * BBLK:(bb + 1) * BBLK]
                )
                return t

            def elementwise(ps, cprev_sb, bb, j, n_split=1, out_ring=None,
                            final=False):
                """Gate nonlinearities + cell update for one quadruple.

                Ops are emitted stage-batched across the splits (all the
                early-ready ACTs first, the o-gate ACTs last) so each
                engine's strict-FIFO queue never blocks a ready op behind a
                not-yet-ready one, and the post-last-matmul chain is only
                ACT(o) + multiply + store of the final split."""
                out_ring = out_ring or nc.gpsimd
                w = BBLK // n_split
                splits = range(n_split)
                csl = [slice(s * w, (s + 1) * w) for s in splits]

                def gate(g, s):
                    at = gpool.tile([128, w], F32, tag=f"act{g}", name=f"act{g}")
                    nc.scalar.activation(
                        at[:], ps[g][:, csl[s]], GATE_FUNCS[g],
                        bias=bias_sb[:, 2 * g + j: 2 * g + j + 1],
                    )
                    return at

                mc, gf, gi, t1, t2, ct, th = ({} for _ in range(7))
                for s in splits:
                    mc[s] = gate(3, s)
                    gf[s] = gate(0, s)
                    gi[s] = gate(1, s)
                for s in splits:
                    t1[s] = gpool.tile([128, w], F32, tag="t1", name="t1")
                    nc.vector.tensor_tensor(
                        t1[s][:], gf[s][:], cprev_sb[:, j, csl[s]], ALU.mult
                    )
                    t2[s] = gpool.tile([128, w], F32, tag="t2", name="t2")
                    nc.vector.tensor_tensor(t2[s][:], gi[s][:], mc[s][:], ALU.mult)
                for s in splits:
                    ct[s] = opool.tile([128, w], F16, tag="ct", name="ct")
                    nc.vector.tensor_tensor(ct[s][:], t1[s][:], t2[s][:], ALU.add)
                    th[s] = gpool.tile([128, w], F32, tag="th", name="th")
                    nc.scalar.activation(th[s][:], ct[s][:], AFT.Tanh)
                for s in splits:
                    osl = slice(bb * BBLK + s * w, bb * BBLK + (s + 1) * w)
                    go = gate(2, s)
                    ht = opool.tile([128, w], F16, tag="ht", name="ht")
                    nc.vector.tensor_tensor(ht[:], go[:], th[s][:], ALU.mult)
                    # final tail: ct on gpsimd, ht on scalar — the ht issue
                    # is the last scalar op so it blocks nothing, and using
                    # the sync ring instead adds a later ring-teardown event
                    # that extends measured exec time.
                    ct_ring = nc.gpsimd if final else out_ring
                    ht_ring = nc.scalar if final else out_ring
                    ct_ring.dma_start(out=cTo_r[:, j, osl], in_=ct[s][:])
                    ht_ring.dma_start(out=hT_r[:, j, osl], in_=ht[:])

            # ---- batch block 0: k-outer over both j's, chasing the DMA
            # stream ----
            cprev0 = load_cprev(0)
            ps0 = [
                [pp.tile([128, BBLK], F32, tag=f"ps{g}", name=f"ps{g}") for g in range(4)]
                for j in range(NJ)
            ]
            for k in range(KT):
                for j in range(NJ):
                    for g in (3, 0, 1, 2):  # c-gate first: longest elementwise chain
                        nc.tensor.matmul(
                            ps0[j][g][:],
                            w_sb[k][:, g * U + j * 128: g * U + (j + 1) * 128],
                            x0_sb[k][:],
                            start=(k == 0),
                            stop=(k == KT - 1),
                        )
            for j in range(NJ):
                elementwise(ps0[j], cprev0, 0, j)

            # ---- batch blocks 1..NBB-1: gate-outer, k-inner ----
            for bb in range(1, NBB):
                bsl = slice(bb * BBLK, (bb + 1) * BBLK)
                # one whole-block x DMA (2MB, 1KB lines): 1 descriptor and 1
                # completion sem per 128 matmuls, issued 2+ blocks ahead
                xt = xpool.tile(
                    [128, KT, BBLK], F16, tag="xq", name="xq", bufs=3
                )
                nc.sync.dma_start(out=xt[:], in_=xT_r[:, :, bsl])
                x_sb = [xt[:, k, :] for k in range(KT)]
                cprev_sb = load_cprev(bb)

                for j in range(NJ):
                    last = (bb == NBB - 1) and (j == NJ - 1)
                    if not last:
                        ps = [None] * 4
                        for g in (3, 0, 1, 2):  # c-gate first: longest chain
                            pst = pp.tile([128, BBLK], F32, tag=f"ps{g}", name=f"ps{g}")
                            col0 = g * U + j * 128
                            for k in range(KT):
                                nc.tensor.matmul(
                                    pst[:],
                                    w_sb[k][:, col0:col0 + 128],
                                    x_sb[k][:],
                                    start=(k == 0),
                                    stop=(k == KT - 1),
                                )
                            ps[g] = pst
                        elementwise(ps, cprev_sb, bb, j)
                    else:
                        # Final quadruple: run each gate as two half-width
                        # accumulation groups (all half-0 gate groups first),
                        # so the half-0 elementwise+DMA overlaps the half-1
                        # matmuls and the post-matmul tail is only one
                        # half-width o-gate ACT + multiply + store.
                        ps = [
                            pp.tile([128, BBLK], F32, tag=f"ps{g}", name=f"ps{g}")
                            for g in range(4)
                        ]
                        # gate order (c, i, f, o): the i-gate feeds the
                        # longest remaining chain (ACT(i) -> t2 -> ct -> th),
                        # which must clear before the o group's 1.75us end.
                        for half in (0, 1):
                            hsl = slice(half * (BBLK // 2), (half + 1) * (BBLK // 2))
                            for g in (3, 1, 0, 2):
                                col0 = g * U + j * 128
                                for k in range(KT):
                                    nc.tensor.matmul(
                                        ps[g][:, hsl],
                                        w_sb[k][:, col0:col0 + 128],
                                        x_sb[k][:, hsl],
                                        start=(k == 0),
                                        stop=(k == KT - 1),
                                    )
                        elementwise(
                            ps, cprev_sb, bb, j,
                            n_split=2, final=True,
                        )

    nc.finalize()
    return nc


def _shard_inputs(e_t, h_prev, c_prev, W_x, b_x, W_h, b_h, b_extra):
    e_t = np.asarray(e_t, dtype=np.float32)
    h_prev = np.asarray(h_prev, dtype=np.float32)
    c_prev = np.ascontiguousarray(np.asarray(c_prev, dtype=np.float32))
    W_x = np.asarray(W_x, dtype=np.float32)
    W_h = np.asarray(W_h, dtype=np.float32)
    bias_full = (
        np.asarray(b_x, dtype=np.float32)
        + np.asarray(b_h, dtype=np.float32)
        + np.asarray(b_extra, dtype=np.float32)
    )

    # X^T = [e_t | h_prev]^T : [K, B] in float16
    XT = np.empty((K, B), dtype=np.float16)
    XT[:D] = e_t.T
    XT[D:] = h_prev.T
    W = np.concatenate([W_x, W_h], axis=1)  # [4H, K] f32

    in_maps = []
    for core in range(M_BATCH * M_UNIT):
        m, q = divmod(core, M_UNIT)
        rows = np.concatenate(
            [np.arange(g0 + q * U, g0 + (q + 1) * U) for g0 in (0, H, 2 * H, 3 * H)]
        )
        in_maps.append({
            "xT": np.ascontiguousarray(XT[:, m * BS:(m + 1) * BS]),
            "wT": np.ascontiguousarray(W[rows].T.astype(np.float16)),
            "bias": np.ascontiguousarray(bias_full[rows]),
            "cT": np.ascontiguousarray(c_prev[m * BS:(m + 1) * BS, q * U:(q + 1) * U].T),
        })
    return in_maps


def _assemble_outputs(results):
    h_t = np.empty((B, H), dtype=np.float32)
    c_t = np.empty((B, H), dtype=np.float32)
    for core, res in enumerate(results):
        m, q = divmod(core, M_UNIT)
        h_t[m * BS:(m + 1) * BS, q * U:(q + 1) * U] = res["hT_out"].T.astype(np.float32)
        c_t[m * BS:(m + 1) * BS, q * U:(q + 1) * U] = res["cT_out"].T.astype(np.float32)
    return h_t, c_t


def kernel(e_t, h_prev, c_prev, W_x, b_x, W_h, b_h, b_extra, _runner=None):
    in_maps = _shard_inputs(e_t, h_prev, c_prev, W_x, b_x, W_h, b_h, b_extra)
    nc = _build_nc()
    if _runner is None:
        res = run_bass_kernel_spmd(nc, in_maps, core_ids=list(range(8)))
        results = res.results
    else:
        results = _runner(nc, in_maps)
    return _assemble_outputs(results)


# revision 32
# speedup vs baseline: 1.0167x; 1.0167x over previous
"""CustomLSTMCell fused kernel for 8x Trainium2 NeuronCores.

Reference computation (B=8192, D=H=1024):
    z = e_t @ W_x.T + h_prev @ W_h.T + (b_x + b_h + b_extra)   # [B, 4H]
    f, i, o, c = split(z, 4)
    c_t = sigmoid(f) * c_prev + sigmoid(i) * tanh(c)
    h_t = sigmoid(o) * tanh(c_t)

Sharding: 2-way batch x 4-way hidden-unit (8 cores, no collectives).
Each core computes z transposed ([gate_rows, batch] layout) so the bias
folds into the ScalarE activation's per-partition bias operand, and both
matmul operands arrive pre-transposed from the host (contraction dim on
partitions).

v2 (this version, ~244-247us HW vs the f32r baseline's ~259us): matmul
operands and h/c outputs in float16 (PSUM accumulation stays fp32;
rel-max-err 6.8e-4, tolerance 2e-2).  HW-measured on this chip: a dense
k-accumulating MATMUL stream runs ~219 ns/MM at 16-bit vs 232.4 ns/MM
at float32r (LDWEIGHTS is hidden by the PE reorder window in both
cases, so the win is the stream rate itself, not weight loads).  fp8
DoubleRow measures 2x (not the cost model's 4x), and plain-e4m3
precision is 4.3x over the error budget (rel-max-err 8.6e-2), so fp8
multi-pass hi/lo schemes all lose to a single 16-bit pass.  fp16 over
bf16 for the extra mantissa.  Moving free dim >512 is rejected by the
ISA (s3d3_mm_num_elements) even at 16-bit.

Measured budget of a good run: ~8us entry butterfly (engines stagger up
0.4-7.4us, DMA queues release ~8.7), warmups to 13.1, 221.5us PE-bound
MM stream (98.6% dense), ~1.5us post-stream elementwise tail, ~4.5us
counted epilogue.  Run-to-run noise is +-2.5us; occasional ~292us runs
are P0 power-state downclock (PE at 2.0GHz), environmental.

Schedule notes:
 - Chase phase: W streams per-k-chunk on the sync HWDGE ring while the
   first batch-block's x streams on gpsimd; the first batch block runs
   its matmuls k-outer so the PE starts as soon as chunk 0 lands and
   chases the arrival stream.  Both rings are up early enough (~2us /
   ~6.2us) to pre-queue descriptors before the DMA queues release
   (~8.7us), and two rings halve the per-chunk delivery time — PE
   chase gaps drop from ~3.3us to ~0 (the scalar ring, up at 7.2us,
   could NOT pre-queue and starved the chase by +3.6us instead).
 - Steady-state x arrives as one whole-block 2MB DMA (1KB lines), one
   completion sem per 128 matmuls, issued 2+ blocks ahead.
 - Each DMA_DIRECT2D costs ~650ns of issue time on its engine, and a DMA
   chain on the scalar engine develops issue-blocking sem waits that
   delay ACTIVATEs (which gate PSUM-bank release).  So: bulk input
   stream on sync, cprev + steady-state outputs on gpsimd (SWDGE), and
   the scalar ring carries only the bias load and the final h store.
 - The final quadruple runs each gate as two half-width accumulation
   groups in (c,i,f,o) order, elementwise ops stage-batched across the
   halves, so the post-last-matmul chain is only ACT(o)+mult+store of
   the second half (~1.5us).
"""

import sys

if "/opt/trn_rl_repo" not in sys.path:
    sys.path.insert(0, "/opt/trn_rl_repo")

import numpy as np

import concourse.bass as bass
import concourse.mybir as mybir
from concourse import bacc
from concourse.bass_utils import run_bass_kernel_spmd
from concourse.tile import TileContext

F32 = mybir.dt.float32
F16 = mybir.dt.float16
AFT = mybir.ActivationFunctionType
ALU = mybir.AluOpType

B, D, H = 8192, 1024, 1024
M_BATCH, M_UNIT = 2, 4          # batch split x unit split = 8 cores
BS = B // M_BATCH               # 4096 batch rows per core
U = H // M_UNIT                 # 256 hidden units per core
K = D + H                       # 2048 contraction (e_t | h_prev)
KT = K // 128                   # 16 k-chunks
G = 4 * U                       # 1024 gate rows per core (f|i|o|c x U)
BBLK = 512                      # moving free-dim per matmul
NBB = BS // BBLK                # 8 batch blocks
NJ = U // 128                   # 2 unit sub-blocks of 128 partitions

GATE_FUNCS = [AFT.Sigmoid, AFT.Sigmoid, AFT.Sigmoid, AFT.Tanh]  # f, i, o, c


def _build_nc():
    nc = bacc.Bacc()

    xT = nc.dram_tensor("xT", [K, BS], F16, kind="ExternalInput")
    wT = nc.dram_tensor("wT", [K, G], F16, kind="ExternalInput")
    bias = nc.dram_tensor("bias", [G], F32, kind="ExternalInput")
    cT = nc.dram_tensor("cT", [U, BS], F32, kind="ExternalInput")
    hT_out = nc.dram_tensor("hT_out", [U, BS], F16, kind="ExternalOutput")
    cT_out = nc.dram_tensor("cT_out", [U, BS], F16, kind="ExternalOutput")

    xT_r = xT.ap().rearrange("(k p) b -> p k b", p=128)      # [128, KT, BS]
    wT_r = wT.ap().rearrange("(k p) g -> p k g", p=128)      # [128, KT, G]
    bias_r = bias.ap().rearrange("(c p) -> p c", p=128)      # [128, 4*NJ]
    cT_r = cT.ap().rearrange("(j p) b -> p j b", p=128)      # [128, NJ, BS]
    hT_r = hT_out.ap().rearrange("(j p) b -> p j b", p=128)
    cTo_r = cT_out.ap().rearrange("(j p) b -> p j b", p=128)

    with TileContext(nc) as tc:
        with (
            tc.tile_pool(name="wpool", bufs=1) as wpool,
            tc.tile_pool(name="xpool", bufs=2) as xpool,
            tc.tile_pool(name="cpool", bufs=2) as cpool,
            tc.tile_pool(name="gpool", bufs=2) as gpool,
            tc.tile_pool(name="opool", bufs=2) as opool,
            tc.tile_pool(name="psum", bufs=2, space="PSUM") as pp,
        ):
            bias_sb = wpool.tile([128, 4 * NJ], F32)
            nc.scalar.dma_start(out=bias_sb[:], in_=bias_r)

            # PE warm-up: ~12 throwaway matmuls on zeroed scratch while the
            # first W/x chunks are in flight (the DMA queues release at
            # ~8.7us after the entry butterfly; w0+x0k0 land ~9.7).  The
            # warmups run 7.9-13.1us, outlasting chunk arrival: a PE-idle
            # gap >3.4us between warmup-end and the stream would re-throttle
            # the HAM clock gate.  (Runs that measure ~292us with a uniform
            # ~263ns/MM stream are P0 power-state downclock to ~2.0GHz —
            # environmental, not schedule-dependent.)
            warm = wpool.tile([128, BBLK], F16, name="warm")
            nc.vector.memset(warm[:], 0.0)
            warm_ps = pp.tile([128, BBLK], F32, tag="ps0", name="warm_ps")
            # 12 warmups (8.0-13.1us) match chunk0's ~12.3us arrival: the
            # pre-queued chase descriptors transfer round-robin across all
            # 16 queues, so chunk0 completes only ~4us after queue release.
            # Fewer warmups (4) measured WORSE (PE idles 9.9-12.5 waiting).
            for _ in range(12):
                nc.tensor.matmul(
                    warm_ps[:], warm[:, 0:128], warm[:], start=True, stop=True
                )

            w_sb = []
            x0_sb = []
            for k in range(KT):
                wt = wpool.tile([128, G], F16, tag=f"w{k}", name=f"w{k}")
                nc.sync.dma_start(out=wt[:], in_=wT_r[:, k, :])
                w_sb.append(wt)
                if k % 2 == 0:
                    xp = xpool.tile(
                        [128, 2, BBLK], F16, tag=f"xp{k // 2}", name=f"xp{k // 2}", bufs=3
                    )
                # x0 on gpsimd (up at ~6.2us — early enough to pre-queue a
                # few descriptors before the queues release at ~8.7us),
                # leaving sync to stream W alone: two rings halve the chase
                # delivery time per k-chunk.  (The scalar ring, up at 7.2,
                # could NOT pre-queue and starved the chase by +3.6us.)
                nc.gpsimd.dma_start(out=xp[:, k % 2, :], in_=xT_r[:, k, 0:BBLK])
                x0_sb.append(xp[:, k % 2, :])

            def load_cprev(bb):
                t = cpool.tile([128, NJ, BBLK], F32, tag="cprev", name="cprev")
                nc.gpsimd.dma_start(
                    out=t[:], in_=cT_r[:, :, bb * BBLK:(bb + 1) * BBLK]
                )
                return t

            def elementwise(ps, cprev_sb, bb, j, n_split=1, out_ring=None,
                            final=False):
                """Gate nonlinearities + cell update for one quadruple.

                Ops are emitted stage-batched across the splits (all the
                early-ready ACTs first, the o-gate ACTs last) so each
                engine's strict-FIFO queue never blocks a ready op behind a
                not-yet-ready one, and the post-last-matmul chain is only
                ACT(o) + multiply + store of the final split."""
                out_ring = out_ring or nc.gpsimd
                w = BBLK // n_split
                splits = range(n_split)
                csl = [slice(s * w, (s + 1) * w) for s in splits]

                def gate(g, s):
                    at = gpool.tile([128, w], F32, tag=f"act{g}", name=f"act{g}")
                    nc.scalar.activation(
                        at[:], ps[g][:, csl[s]], GATE_FUNCS[g],
                        bias=bias_sb[:, 2 * g + j: 2 * g + j + 1],
                    )
                    return at

                mc, gf, gi, t1, t2, ct, th = ({} for _ in range(7))
                for s in splits:
                    mc[s] = gate(3, s)
                    gf[s] = gate(0, s)
                    gi[s] = gate(1, s)
                for s in splits:
                    t1[s] = gpool.tile([128, w], F32, tag="t1", name="t1")
                    nc.vector.tensor_tensor(
                        t1[s][:], gf[s][:], cprev_sb[:, j, csl[s]], ALU.mult
                    )
                    t2[s] = gpool.tile([128, w], F32, tag="t2", name="t2")
                    nc.vector.tensor_tensor(t2[s][:], gi[s][:], mc[s][:], ALU.mult)
                for s in splits:
                    ct[s] = opool.tile([128, w], F16, tag="ct", name="ct")
                    nc.vector.tensor_tensor(ct[s][:], t1[s][:], t2[s][:], ALU.add)
                    th[s] = gpool.tile([128, w], F32, tag="th", name="th")
                    nc.scalar.activation(th[s][:], ct[s][:], AFT.Tanh)
                for s in splits:
                    osl = slice(bb * BBLK + s * w, bb * BBLK + (s + 1) * w)
                    go = gate(2, s)
                    ht = opool.tile([128, w], F16, tag="ht", name="ht")
                    nc.vector.tensor_tensor(ht[:], go[:], th[s][:], ALU.mult)
                    # final tail: ct on gpsimd, ht on scalar — the ht issue
                    # is the last scalar op so it blocks nothing, and using
                    # the sync ring instead adds a later ring-teardown event
                    # that extends measured exec time.
                    ct_ring = nc.gpsimd if final else out_ring
                    ht_ring = nc.scalar if final else out_ring
                    ct_ring.dma_start(out=cTo_r[:, j, osl], in_=ct[s][:])
                    ht_ring.dma_start(out=hT_r[:, j, osl], in_=ht[:])

            # ---- batch block 0: k-outer over both j's, chasing the DMA
            # stream ----
            cprev0 = load_cprev(0)
            ps0 = [
                [pp.tile([128, BBLK], F32, tag=f"ps{g}", name=f"ps{g}") for g in range(4)]
                for j in range(NJ)
            ]
            for k in range(KT):
                for j in range(NJ):
                    for g in (3, 0, 1, 2):  # c-gate first: longest elementwise chain
                        nc.tensor.matmul(
                            ps0[j][g][:],
                            w_sb[k][:, g * U + j * 128: g * U + (j + 1) * 128],
                            x0_sb[k][:],
                            start=(k == 0),
                            stop=(k == KT - 1),
                        )
            for j in range(NJ):
                elementwise(ps0[j], cprev0, 0, j)

            # ---- batch blocks 1..NBB-1: gate-outer, k-inner ----
            for bb in range(1, NBB):
                bsl = slice(bb * BBLK, (bb + 1) * BBLK)
                # one whole-block x DMA (2MB, 1KB lines): 1 descriptor and 1
                # completion sem per 128 matmuls, issued 2+ blocks ahead
                xt = xpool.tile(
                    [128, KT, BBLK], F16, tag="xq", name="xq", bufs=3
                )
                nc.sync.dma_start(out=xt[:], in_=xT_r[:, :, bsl])
                x_sb = [xt[:, k, :] for k in range(KT)]
                cprev_sb = load_cprev(bb)

                for j in range(NJ):
                    last = (bb == NBB - 1) and (j == NJ - 1)
                    if not last:
                        ps = [None] * 4
                        for g in (3, 0, 1, 2):  # c-gate first: longest chain
                            pst = pp.tile([128, BBLK], F32, tag=f"ps{g}", name=f"ps{g}")
                            col0 = g * U + j * 128
                            for k in range(KT):
                                nc.tensor.matmul(
                                    pst[:],
                                    w_sb[k][:, col0:col0 + 128],
                                    x_sb[k][:],
                                    start=(k == 0),
                                    stop=(k == KT - 1),
                                )
                            ps[g] = pst
                        elementwise(ps, cprev_sb, bb, j)
                    else:
                        # Final quadruple: run each gate as two half-width
                        # accumulation groups (all half-0 gate groups first),
                        # so the half-0 elementwise+DMA overlaps the half-1
                        # matmuls and the post-matmul tail is only one
                        # half-width o-gate ACT + multiply + store.
                        ps = [
                            pp.tile([128, BBLK], F32, tag=f"ps{g}", name=f"ps{g}")
                            for g in range(4)
                        ]
                        # gate order (c, i, f, o): the i-gate feeds the
                        # longest remaining chain (ACT(i) -> t2 -> ct -> th),
                        # which must clear before the o group's 1.75us end.
                        for half in (0, 1):
                            hsl = slice(half * (BBLK // 2), (half + 1) * (BBLK // 2))
                            for g in (3, 1, 0, 2):
                                col0 = g * U + j * 128
                                for k in range(KT):
                                    nc.tensor.matmul(
                                        ps[g][:, hsl],
                                        w_sb[k][:, col0:col0 + 128],
                                        x_sb[k][:, hsl],
                                        start=(k == 0),
                                        stop=(k == KT - 1),
                                    )
                        elementwise(
                            ps, cprev_sb, bb, j,
                            n_split=2, final=True,
                        )

    nc.finalize()
    return nc


def _shard_inputs(e_t, h_prev, c_prev, W_x, b_x, W_h, b_h, b_extra):
    e_t = np.asarray(e_t, dtype=np.float32)
    h_prev = np.asarray(h_prev, dtype=np.float32)
    c_prev = np.ascontiguousarray(np.asarray(c_prev, dtype=np.float32))
    W_x = np.asarray(W_x, dtype=np.float32)
    W_h = np.asarray(W_h, dtype=np.float32)
    bias_full = (
        np.asarray(b_x, dtype=np.float32)
        + np.asarray(b_h, dtype=np.float32)
        + np.asarray(b_extra, dtype=np.float32)
    )

    # X^T = [e_t | h_prev]^T : [K, B] in float16
    XT = np.empty((K, B), dtype=np.float16)
    XT[:D] = e_t.T
    XT[D:] = h_prev.T
    W = np.concatenate([W_x, W_h], axis=1)  # [4H, K] f32

    in_maps = []
    for core in range(M_BATCH * M_UNIT):
        m, q = divmod(core, M_UNIT)
        rows = np.concatenate(
            [np.arange(g0 + q * U, g0 + (q + 1) * U) for g0 in (0, H, 2 * H, 3 * H)]
        )
        in_maps.append({
            "xT": np.ascontiguousarray(XT[:, m * BS:(m + 1) * BS]),
            "wT": np.ascontiguousarray(W[rows].T.astype(np.float16)),
            "bias": np.ascontiguousarray(bias_full[rows]),
            "cT": np.ascontiguousarray(c_prev[m * BS:(m + 1) * BS, q * U:(q + 1) * U].T),
        })
    return in_maps


def _assemble_outputs(results):
    h_t = np.empty((B, H), dtype=np.float32)
    c_t = np.empty((B, H), dtype=np.float32)
    for core, res in enumerate(results):
        m, q = divmod(core, M_UNIT)
        h_t[m * BS:(m + 1) * BS, q * U:(q + 1) * U] = res["hT_out"].T.astype(np.float32)
        c_t[m * BS:(m + 1) * BS, q * U:(q + 1) * U] = res["cT_out"].T.astype(np.float32)
    return h_t, c_t


def kernel(e_t, h_prev, c_prev, W_x, b_x, W_h, b_h, b_extra, _runner=None):
    in_maps = _shard_inputs(e_t, h_prev, c_prev, W_x, b_x, W_h, b_h, b_extra)
    nc = _build_nc()
    if _runner is None:
        res = run_bass_kernel_spmd(nc, in_maps, core_ids=list(range(8)))
        results = res.results
    else:
        results = _runner(nc, in_maps)
    return _assemble_outputs(results)
